# revision 28
# baseline (speedup 1.0000x reference)
"""Trainium2 Bass kernel for LinearWaveAttention (B=4, S=4096, H=1024, 16 heads, D=64).

Sharding: 8 cores = 4 batches x 2 head-groups (8 heads each). Each core computes
its batch's projections restricted to its heads, the wave feature map, the causal
linear-attention scan (chunked, 128 positions per chunk), and a partial output
projection against its Wo row-block. Host sums the two partials per batch + bo.

All projection/output matmuls run in float32r (fast fp32 mode); attention
internals run in bf16 with fp32 PSUM accumulation.

Feature map identity used: elu(amp)+1 == amp+1 (amp >= 0), and
cos(sin(atan2(i, r))) == cos(i / sqrt(r^2 + i^2)) evaluated with an even cubic
polynomial in s = i^2/(r^2+i^2) (max err 2.6e-7).
"""
import sys
sys.path.insert(0, "/opt/trn_rl_repo")
import numpy as np

HIDDEN = 1024
NH_LOC = 8            # heads per core
D = 64
S = 4096
B = 4
CP = 512              # projection super-chunk (seq positions)
C = 128               # attention chunk
NCP = S // CP         # 8
NSUB = CP // C        # 4
N_CORES = 8
EPS = 1e-6

# cos(u) ~ ((s + BETA/2)^2 + DELTA) * (C3*s - C3*R1), s = u^2 in [0,1]
C3 = -0.001340061216981847
R1 = 2.4663718399873695
BETA_HALF = -14.30231189756757
DELTA = 98.00754010828652

_CACHE = {}


def _build(reps: int = 1):
    import concourse.tile as tile
    from concourse import bacc, mybir
    from concourse.masks import make_identity, make_upper_triangular
    from contextlib import ExitStack

    dt = mybir.dt
    AF = mybir.ActivationFunctionType
    OP = mybir.AluOpType

    nc = bacc.Bacc("TRN2", target_bir_lowering=False, debug=False)
    xT = nc.declare_dram_parameter("xT", [2 * HIDDEN, S], dt.float32r, isOutput=False)
    wq_d = nc.declare_dram_parameter("wq", [HIDDEN, 512], dt.float32r, isOutput=False)
    wk_d = nc.declare_dram_parameter("wk", [HIDDEN, 512], dt.float32r, isOutput=False)
    wv_d = nc.declare_dram_parameter("wv", [HIDDEN, 512], dt.float32r, isOutput=False)
    wo_d = nc.declare_dram_parameter("wo", [512, HIDDEN], dt.float32r, isOutput=False)
    out_d = nc.declare_dram_parameter("out", [S, HIDDEN], dt.float32, isOutput=True)

    with tile.TileContext(nc) as tc, ExitStack() as ctx:
        wpool = ctx.enter_context(tc.tile_pool(name="w", bufs=1))
        xpool = ctx.enter_context(tc.tile_pool(name="x", bufs=2))
        feat = ctx.enter_context(tc.tile_pool(name="feat", bufs=1))
        qkv = ctx.enter_context(tc.tile_pool(name="qkv", bufs=2))
        opool = ctx.enter_context(tc.tile_pool(name="o", bufs=2))
        cpool = ctx.enter_context(tc.tile_pool(name="c", bufs=1))
        ps_state = ctx.enter_context(tc.tile_pool(name="pstate", bufs=1, space="PSUM"))
        ps_proj = ctx.enter_context(tc.tile_pool(name="pproj", bufs=3, space="PSUM"))
        ps_attn = ctx.enter_context(tc.tile_pool(name="pattn", bufs=4, space="PSUM"))

        # ---- constants ----
        ident = cpool.tile([128, 128], dt.bfloat16, tag="ident")
        make_identity(nc, ident)
        mask4 = cpool.tile([128, 512], dt.float32, tag="mask4")
        for j in range(4):
            make_upper_triangular(nc, mask4[:, 128 * j:128 * (j + 1)], val=1.0, diag=True)
        zrow = cpool.tile([1, 64], dt.bfloat16, tag="zrow")
        nc.vector.memset(zrow, 0.0)
        eps_b = cpool.tile([128, 1], dt.float32, tag="eps_b")
        nc.vector.memset(eps_b, EPS)
        beta_b = cpool.tile([128, 1], dt.float32, tag="beta_b")
        nc.vector.memset(beta_b, BETA_HALF)

        # ---- weights ----
        wq = wpool.tile([128, 8, 512], dt.float32r, tag="wq")
        wk = wpool.tile([128, 8, 512], dt.float32r, tag="wk")
        wv = wpool.tile([128, 8, 512], dt.float32r, tag="wv")
        wo = wpool.tile([128, 4, 1024], dt.float32r, tag="wo")
        for k in range(8):
            nc.sync.dma_start(out=wq[:, k, :], in_=wq_d[128 * k:128 * (k + 1), :])

        def load_rest_weights():
            for k in range(8):
                nc.sync.dma_start(out=wk[:, k, :], in_=wk_d[128 * k:128 * (k + 1), :])
            for k in range(8):
                nc.sync.dma_start(out=wv[:, k, :], in_=wv_d[128 * k:128 * (k + 1), :])
            for k in range(4):
                nc.sync.dma_start(out=wo[:, k, :], in_=wo_d[128 * k:128 * (k + 1), :])

        # ---- persistent attention state: head h at [0:64, 64h:64h+64] ----
        state = ps_state.tile([64, 512], dt.float32)
        nc.tensor.matmul(state[:, :], zrow[0:1, 0:64], zrow[0:1, 0:1].broadcast_to((1, 512)),
                         start=True, stop=False, skip_group_check=True)

        def feature_evict(pr, pi):
            """Evict projection PSUM pair as squares (frees proj slots fast)."""
            A = feat.tile([128, 512], dt.float32, tag="fA", bufs=2)
            Bt = feat.tile([128, 512], dt.float32, tag="fB", bufs=2)
            nc.scalar.activation(out=A, in_=pr, func=AF.Square)
            nc.scalar.activation(out=Bt, in_=pi, func=AF.Square)
            return A, Bt

        def feature_rest(A, Bt, out_ap):
            """out = (1+sqrt(t+EPS)) * cos_poly(B/t), t = A+B. [128,512]."""
            t = feat.tile([128, 512], dt.float32, tag="ft")
            nc.gpsimd.tensor_add(out=t, in0=A, in1=Bt)
            amp = feat.tile([128, 512], dt.float32, tag="famp")
            nc.scalar.activation(out=amp, in_=t, func=AF.Sqrt, bias=eps_b)
            it = feat.tile([128, 512], dt.float32, tag="fit")
            nc.vector.reciprocal_approx_fast(out=it, in_=t)
            ss = feat.tile([128, 512], dt.float32, tag="fss")
            nc.gpsimd.tensor_mul(out=ss, in0=Bt, in1=it)
            w2 = feat.tile([128, 512], dt.float32, tag="fw2")
            nc.scalar.activation(out=w2, in_=ss, func=AF.Square, bias=beta_b)
            nc.vector.tensor_scalar(out=ss, in0=ss, scalar1=C3, scalar2=C3 * R1,
                                    op0=OP.mult, op1=OP.subtract)
            nc.vector.scalar_tensor_tensor(out=w2, in0=w2, scalar=DELTA, in1=ss,
                                           op0=OP.add, op1=OP.mult)
            nc.vector.scalar_tensor_tensor(out=out_ap, in0=amp, scalar=1.0, in1=w2,
                                           op0=OP.add, op1=OP.mult)

        def stage_proj(ci, load_w=False):
            c0 = CP * ci
            # ---- load xT chunk: ktiles 0-7 real, 8-15 imag ----
            xt = xpool.tile([128, 16, 512], dt.float32r, tag="xt")
            for k in range(16):
                nc.sync.dma_start(out=xt[:, k, :],
                                  in_=xT[128 * k:128 * (k + 1), c0:c0 + CP])
            if load_w:
                load_rest_weights()

            # ---- transposed projections + feature -> Qt / Kt (2 heads per group) ----
            qt = qkv.tile([128, 4, 512], dt.bfloat16, tag="qt")
            kt = qkv.tile([128, 4, 512], dt.bfloat16, tag="kt")
            for (wmat, dest) in ((wq, qt), (wk, kt)):
                pending = []
                for j in range(4):
                    # pr/pi matmuls interleaved so consecutive instructions
                    # share the same stationary operand (LDW pairing)
                    pr = ps_proj.tile([128, 512], dt.float32, tag="proj")
                    pi = ps_proj.tile([128, 512], dt.float32, tag="proj")
                    for k in range(8):
                        nc.tensor.matmul(pr, wmat[:, k, 128 * j:128 * (j + 1)],
                                         xt[:, k, :], start=(k == 0), stop=(k == 7))
                        nc.tensor.matmul(pi, wmat[:, k, 128 * j:128 * (j + 1)],
                                         xt[:, k + 8, :], start=(k == 0), stop=(k == 7))
                    pending.append((feature_evict(pr, pi), j))
                for (A, Bt), j in pending:
                    feature_rest(A, Bt, dest[:, j, :])
            # odd heads shifted to base partition 0 (engines can't cross partitions; DMA can)
            qt_o = qkv.tile([64, 4, 512], dt.bfloat16, tag="qto")
            kt_o = qkv.tile([64, 4, 512], dt.bfloat16, tag="kto")
            for j in range(4):
                nc.gpsimd.dma_start(out=qt_o[:, j, :], in_=qt[64:128, j, :])
                nc.gpsimd.dma_start(out=kt_o[:, j, :], in_=kt[64:128, j, :])

            # ---- V projections (normal layout, per sub-chunk) ----
            v3 = qkv.tile([128, 4, 512], dt.bfloat16, tag="v3")
            for s in range(NSUB):
                pv = ps_proj.tile([128, 512], dt.float32, tag="proj")
                for k in range(8):
                    nc.tensor.matmul(pv, xt[:, k, 128 * s:128 * (s + 1)],
                                     wv[:, k, :], start=(k == 0), stop=(k == 7))
                if s % 2 == 0:
                    nc.scalar.activation(out=v3[:, s, :], in_=pv, func=AF.Copy)
                else:
                    nc.vector.tensor_copy(out=v3[:, s, :], in_=pv)
            return qt, kt, qt_o, kt_o, v3

        def stage_attn(ci, qt, kt, qt_o, kt_o, v3):
            def qt_slice(h, s):
                j, par = h // 2, h % 2
                src = qt_o if par else qt
                return src[0:64, j, 128 * s:128 * (s + 1)]

            def kt_slice(h, s):
                j, par = h // 2, h % 2
                src = kt_o if par else kt
                return src[0:64, j, 128 * s:128 * (s + 1)]

            # ---- attention + output projection per sub-chunk ----
            for s in range(NSUB):
                gchunk = NSUB * ci + s
                first = gchunk == 0
                last = gchunk == S // C - 1

                # K normal layout via DMA XBAR transpose (keeps PE free)
                kn = qkv.tile([128, 512], dt.bfloat16, tag="kn_sb", bufs=2)
                for j in range(4):
                    nc.scalar.dma_start(out=kn[:, 128 * j:128 * (j + 1)],
                                      in_=kt[:, j, 128 * s:128 * (s + 1)],
                                      transpose=True)

                if not first:
                    s_sb = qkv.tile([64, 512], dt.bfloat16, tag="s_sb", bufs=2)
                    nc.scalar.activation(out=s_sb, in_=state, func=AF.Copy)

                at_tiles = []
                for tb in range(2):
                    tps = ps_attn.tile([128, 512], dt.float32, tag="attn")
                    for hh in range(4):
                        h = 4 * tb + hh
                        nc.tensor.matmul(tps[:, 128 * hh:128 * (hh + 1)],
                                         kt_slice(h, s), qt_slice(h, s),
                                         start=True, stop=True)
                    at = qkv.tile([128, 512], dt.bfloat16, tag="at", bufs=3)
                    nc.vector.tensor_tensor(out=at, in0=tps, in1=mask4, op=OP.mult)
                    at_tiles.append(at)

                otp = ps_attn.tile([128, 512], dt.float32, tag="attn")
                for h in range(NH_LOC):
                    par, col = 64 * (h % 2), 128 * (h // 2)
                    slot = otp[par:par + 64, col:col + 128]
                    at = at_tiles[h // 4][:, 128 * (h % 4):128 * (h % 4 + 1)]
                    nc.tensor.matmul(slot, v3[:, s, 64 * h:64 * (h + 1)], at,
                                     start=True, stop=first, tile_position=(0, par))
                    if not first:
                        nc.tensor.matmul(slot, s_sb[0:64, 64 * h:64 * (h + 1)],
                                         qt_slice(h, s), start=False, stop=True,
                                         tile_position=(0, par))
                    # state += K_chunk^T V_chunk (after s_sb snapshot)
                    kcol = 128 * (h // 2) + 64 * (h % 2)
                    nc.tensor.matmul(state[0:64, 64 * h:64 * (h + 1)],
                                     kn[:, kcol:kcol + 64],
                                     v3[:, s, 64 * h:64 * (h + 1)],
                                     start=False, stop=(last and h == NH_LOC - 1),
                                     skip_group_check=True)

                ot_sb = opool.tile([128, 512], dt.float32r, tag="ot_sb", bufs=2)
                nc.scalar.activation(out=ot_sb, in_=otp, func=AF.Copy)

                out_sb = opool.tile([128, 1024], dt.float32, tag="out_sb")
                for half in range(2):
                    op_ps = ps_proj.tile([128, 512], dt.float32, tag="proj")
                    for kk in range(4):
                        nc.tensor.matmul(op_ps, ot_sb[:, 128 * kk:128 * (kk + 1)],
                                         wo[:, kk, 512 * half:512 * (half + 1)],
                                         start=(kk == 0), stop=(kk == 3))
                    if half == 0:
                        nc.scalar.activation(out=out_sb[:, 0:512], in_=op_ps, func=AF.Copy)
                    else:
                        nc.vector.tensor_copy(out=out_sb[:, 512:1024], in_=op_ps)
                row = C * gchunk
                nc.gpsimd.dma_start(out=out_d[row:row + C, :], in_=out_sb)

        prev = None  # (ci, proj tiles)
        for rep in range(reps):
            if prev is not None:
                # flush last chunk of previous rep, then re-zero the scan state
                stage_attn(prev[0], *prev[1])
                prev = None
                nc.tensor.matmul(state[:, :], zrow[0:1, 0:64],
                                 zrow[0:1, 0:1].broadcast_to((1, 512)),
                                 start=True, stop=False, skip_group_check=True)
            for ci in range(NCP):
                cur = (ci, stage_proj(ci, load_w=(rep == 0 and ci == 0)))
                if prev is not None:
                    stage_attn(prev[0], *prev[1])
                prev = cur
        stage_attn(prev[0], *prev[1])

    nc.compile()
    return nc


def _get_nc(reps: int = 1):
    if reps not in _CACHE:
        _CACHE[reps] = _build(reps)
    return _CACHE[reps]


def make_in_maps(wave_hidden_states, Wq, Wk, Wv, Wo, bo=None):
    """Per-core input dicts: core = 2*batch + head_group."""
    wave = np.ascontiguousarray(np.asarray(wave_hidden_states, dtype=np.float32))
    Wq = np.asarray(Wq, dtype=np.float32)
    Wk = np.asarray(Wk, dtype=np.float32)
    Wv = np.asarray(Wv, dtype=np.float32)
    Wo = np.asarray(Wo, dtype=np.float32)
    in_maps = []
    for core in range(N_CORES):
        b, g = core // 2, core % 2
        cols = slice(512 * g, 512 * (g + 1))
        in_maps.append({
            "xT": np.ascontiguousarray(wave[b].T),          # [2048, S]
            "wq": np.ascontiguousarray(Wq[:, cols]),
            "wk": np.ascontiguousarray(Wk[:, cols]),
            "wv": np.ascontiguousarray(Wv[:, cols]),
            "wo": np.ascontiguousarray(Wo[cols, :]),
        })
    return in_maps


def kernel(wave_hidden_states, Wq, Wk, Wv, Wo, bo):
    from concourse.bass_utils import run_bass_kernel_spmd

    nc = _get_nc()
    bo = np.asarray(bo, dtype=np.float32)
    in_maps = make_in_maps(wave_hidden_states, Wq, Wk, Wv, Wo)
    res = run_bass_kernel_spmd(nc, in_maps, core_ids=list(range(N_CORES)))
    out = np.empty((B, S, HIDDEN), dtype=np.float32)
    for b in range(B):
        out[b] = res.results[2 * b]["out"] + res.results[2 * b + 1]["out"] + bo
    return out

